# revision 27
# baseline (speedup 1.0000x reference)
"""Trainium2 Bass kernel for nn_EnsembleLinear (block-diagonal ensemble linear).

Full-input contract: kernel(x, W) -> out with
    x  [4096, 8192] f32
    W  [8192, 8192] f32 (only the 32 diagonal 256x256 blocks matter)
    out[4096, 8192] f32,  out[b, d*256+o] = sum_i W[d*256+o, d*256+i] * x[b, d*256+i]

Sharding: detectors (block axis) split across 8 cores, 4 detectors/core.
Host prep transposes each core's x slice and extracts/transposes the W
diagonal blocks so the device kernel is a pure stationary-weight GEMM
with no on-chip transposes. Output is produced transposed ([o, b]) and
un-transposed on the host.
"""

import numpy as np

B = 4096          # batch
NCORES = 8
DPC = 4           # detectors per core
BLK = 256         # block size (both in/out features per detector)
P = 128           # partitions
KC = BLK // P     # k chunks per block (2)
OC = BLK // P     # o chunks per block (2)
MMB = 512         # batch columns per matmul (max fp32 moving free dim)
BGRP = 1024       # batch columns per compute stage
XGRP = 4096       # batch columns per out tile (8KB bf16 lines)
NBH = BGRP // MMB # matmul slices per stage
NBG = B // BGRP   # compute groups
NXI = XGRP // BGRP  # compute groups per out tile
FPC = DPC * BLK   # features per core (1024)

# Matmul input dtype: "float32" (exact, 4 cyc/row), "float32r" (1 cyc/row),
# or "bfloat16" (1 cyc/row, halves input DMA bytes).
MM_DTYPE = "bfloat16"
# Output DMA dtype: "float32" (exact) or "bfloat16" (halves output DMA bytes).
OUT_DTYPE = "bfloat16"


def _build_bass():
    import concourse.mybir as mybir
    import concourse.tile as tile
    from concourse import bacc

    mm_dt = getattr(mybir.dt, MM_DTYPE)
    out_dt = getattr(mybir.dt, OUT_DTYPE)
    nc = bacc.Bacc("TRN2", target_bir_lowering=False, debug=False,
                   num_devices=NCORES)
    xT = nc.dram_tensor("xT", [FPC, B], mm_dt, kind="ExternalInput")
    wT = nc.dram_tensor("wT", [P, DPC * KC * OC * P], mm_dt, kind="ExternalInput")
    outT = nc.dram_tensor("outT", [FPC, B], out_dt, kind="ExternalOutput")

    with tile.TileContext(nc) as tc:
        with (
            tc.tile_pool(name="wpool", bufs=1) as wpool,
            tc.tile_pool(name="xpool", bufs=1) as xpool,
            tc.tile_pool(name="opool", bufs=1) as opool,
            tc.tile_pool(name="psum", bufs=4, space="PSUM") as psum,
        ):
            # Load ALL of x up front (8KB/partition lines, one DMA per
            # i-chunk). The whole per-core x fits in SBUF at 2-byte dtypes,
            # so no slab pipelining is needed and the PE never waits on a
            # mid-kernel input stage. The first two chunks are split in half
            # and issued before the weights so the first matmuls start as
            # early as possible.
            xT_v = xT.rearrange("(c p) b -> p c b", p=P)  # [128, 8, 4096]
            xs = []
            for ci in range(DPC * KC):
                xc = xpool.tile([P, B], mm_dt, tag=f"xc{ci}")
                xs.append(xc)
            H = B // 2
            nc.sync.dma_start(xs[0][:, :H], xT_v[:, 0, :H])
            nc.sync.dma_start(xs[1][:, :H], xT_v[:, 1, :H])
            # Stationary weights, resident for the whole kernel:
            # w_sb[i, (d,kc,oc), o]
            w_sb = wpool.tile([P, DPC * KC * OC, P], mm_dt)
            nc.sync.dma_start(w_sb[:], wT.rearrange("p (c o) -> p c o", o=P))
            nc.sync.dma_start(xs[0][:, H:], xT_v[:, 0, H:])
            nc.sync.dma_start(xs[1][:, H:], xT_v[:, 1, H:])
            for ci in range(2, DPC * KC):
                nc.sync.dma_start(xs[ci][:], xT_v[:, ci, :])
            # d-major compute order: detector d only needs x chunks 2d and
            # 2d+1, which are exactly the chunks that have already arrived
            # when its matmuls are reached — the PE never waits mid-kernel.
            for d in range(DPC):
                for oc in range(OC):
                    ot = None
                    for bg in range(NBG):
                        po = psum.tile([P, BGRP], mybir.dt.float32)
                        for kc in range(KC):
                            ci = d * KC + kc
                            wi = (d * KC + kc) * OC + oc
                            for bh in range(NBH):
                                b0 = bg * BGRP + bh * MMB
                                nc.tensor.matmul(
                                    po[:, bh * MMB:(bh + 1) * MMB],
                                    w_sb[:, wi, :],          # lhsT [i, o]
                                    xs[ci][:, b0:b0 + MMB],
                                    start=(kc == 0),
                                    stop=(kc == KC - 1),
                                )
                        # Out tiles span XGRP batch columns so out-DMA lines
                        # stay >=4KB even at 2-byte output dtypes.
                        if bg % NXI == 0:
                            ot = opool.tile(
                                [P, XGRP], out_dt, tag=f"ot{d}_{oc}",
                                name=f"ot{d}_{oc}")
                        otv = ot[:, (bg % NXI) * BGRP:(bg % NXI + 1) * BGRP]
                        # Split each evacuation across DVE and ACT so PSUM
                        # banks free sooner and both engines share the load.
                        nc.vector.tensor_copy(otv[:, :MMB], po[:, :MMB])
                        nc.scalar.copy(otv[:, MMB:], po[:, MMB:])
                        if bg % NXI == NXI - 1:
                            row0 = d * BLK + oc * P
                            nc.sync.dma_start(
                                outT[row0:row0 + P,
                                     (bg - NXI + 1) * BGRP:
                                     (bg + 1) * BGRP],
                                ot[:])
    nc.compile()
    return nc


def _np_dt(name):
    if name == "bfloat16":
        import ml_dtypes
        return np.dtype(ml_dtypes.bfloat16)
    return np.dtype(np.float32)


def _shard_inputs(x: np.ndarray, W: np.ndarray):
    """Per-core input maps: transposed x slice + transposed diag W chunks."""
    io_dt = _np_dt(MM_DTYPE)
    in_maps = []
    for c in range(NCORES):
        xs = x[:, c * FPC:(c + 1) * FPC]
        xT = np.ascontiguousarray(xs.T).astype(io_dt)        # [1024, 4096]
        wblk = np.empty((P, DPC * KC * OC, P), np.float32)   # [i, (d,kc,oc), o]
        for d in range(DPC):
            gd = c * DPC + d
            Wb = W[gd * BLK:(gd + 1) * BLK, gd * BLK:(gd + 1) * BLK]  # [o, i]
            for kc in range(KC):
                for oc in range(OC):
                    wblk[:, (d * KC + kc) * OC + oc, :] = \
                        Wb[oc * P:(oc + 1) * P, kc * P:(kc + 1) * P].T
        in_maps.append({"xT": xT, "wT": np.ascontiguousarray(
            wblk.reshape(P, DPC * KC * OC * P)).astype(io_dt)})
    return in_maps


def _unshard_output(results):
    out = np.empty((B, NCORES * FPC), np.float32)
    for c in range(NCORES):
        out[:, c * FPC:(c + 1) * FPC] = results[c]["outT"].T.astype(np.float32)
    return out


_NC_CACHE = None


def kernel(x: np.ndarray, W: np.ndarray) -> np.ndarray:
    import time
    from concourse.bass_utils import run_bass_kernel_spmd

    global _NC_CACHE
    if _NC_CACHE is None:
        _NC_CACHE = _build_bass()
    in_maps = _shard_inputs(np.asarray(x, np.float32), np.asarray(W, np.float32))
    last_exc = None
    for attempt in range(3):
        try:
            res = run_bass_kernel_spmd(
                _NC_CACHE, in_maps, core_ids=list(range(NCORES)))
            return _unshard_output(res.results)
        except Exception as e:  # transient NRT/device errors: retry
            last_exc = e
            time.sleep(2.0)
    raise last_exc


# revision 28
# speedup vs baseline: 1.0122x; 1.0122x over previous
"""Trainium2 Bass kernel for nn_EnsembleLinear (block-diagonal ensemble linear).

Full-input contract: kernel(x, W) -> out with
    x  [4096, 8192] f32
    W  [8192, 8192] f32 (only the 32 diagonal 256x256 blocks matter)
    out[4096, 8192] f32,  out[b, d*256+o] = sum_i W[d*256+o, d*256+i] * x[b, d*256+i]

Sharding: detectors (block axis) split across 8 cores, 4 detectors/core.
Host prep transposes each core's x slice and extracts/transposes the W
diagonal blocks so the device kernel is a pure stationary-weight GEMM
with no on-chip transposes. Output is produced transposed ([o, b]) and
un-transposed on the host.
"""

import numpy as np

B = 4096          # batch
NCORES = 8
DPC = 4           # detectors per core
BLK = 256         # block size (both in/out features per detector)
P = 128           # partitions
KC = BLK // P     # k chunks per block (2)
OC = BLK // P     # o chunks per block (2)
MMB = 512         # batch columns per matmul (max fp32 moving free dim)
BGRP = 1024       # batch columns per compute stage
XGRP = 4096       # batch columns per out tile (8KB bf16 lines)
NBH = BGRP // MMB # matmul slices per stage
NBG = B // BGRP   # compute groups
NXI = XGRP // BGRP  # compute groups per out tile
FPC = DPC * BLK   # features per core (1024)

# Matmul input dtype: "float32" (exact, 4 cyc/row), "float32r" (1 cyc/row),
# or "bfloat16" (1 cyc/row, halves input DMA bytes).
MM_DTYPE = "bfloat16"
# Output DMA dtype: "float32" (exact) or "bfloat16" (halves output DMA bytes).
OUT_DTYPE = "bfloat16"


def _build_bass():
    import concourse.mybir as mybir
    import concourse.tile as tile
    from concourse import bacc

    mm_dt = getattr(mybir.dt, MM_DTYPE)
    out_dt = getattr(mybir.dt, OUT_DTYPE)
    nc = bacc.Bacc("TRN2", target_bir_lowering=False, debug=False,
                   num_devices=NCORES)
    xT = nc.dram_tensor("xT", [FPC, B], mm_dt, kind="ExternalInput")
    wT = nc.dram_tensor("wT", [P, DPC * KC * OC * P], mm_dt, kind="ExternalInput")
    outT = nc.dram_tensor("outT", [FPC, B], out_dt, kind="ExternalOutput")

    with tile.TileContext(nc) as tc:
        with (
            tc.tile_pool(name="wpool", bufs=1) as wpool,
            tc.tile_pool(name="xpool", bufs=1) as xpool,
            tc.tile_pool(name="opool", bufs=1) as opool,
            tc.tile_pool(name="psum", bufs=4, space="PSUM") as psum,
        ):
            # Stationary weights, resident for the whole kernel:
            # w_sb[i, (d,kc,oc), o]
            w_sb = wpool.tile([P, DPC * KC * OC, P], mm_dt)
            nc.sync.dma_start(w_sb[:], wT.rearrange("p (c o) -> p c o", o=P))

            # Load ALL of x up front (8KB/partition lines, one DMA per
            # i-chunk). The whole per-core x fits in SBUF at 2-byte dtypes,
            # so no slab pipelining is needed and the PE never waits on a
            # mid-kernel input stage.
            xT_v = xT.rearrange("(c p) b -> p c b", p=P)  # [128, 8, 4096]
            xs = []
            for ci in range(DPC * KC):
                xc = xpool.tile([P, B], mm_dt, tag=f"xc{ci}")
                nc.sync.dma_start(xc[:], xT_v[:, ci, :])
                xs.append(xc)
            # d-major compute order: detector d only needs x chunks 2d and
            # 2d+1, which are exactly the chunks that have already arrived
            # when its matmuls are reached — the PE never waits mid-kernel.
            for d in range(DPC):
                for oc in range(OC):
                    ot = None
                    for bg in range(NBG):
                        po = psum.tile([P, BGRP], mybir.dt.float32)
                        for kc in range(KC):
                            ci = d * KC + kc
                            wi = (d * KC + kc) * OC + oc
                            for bh in range(NBH):
                                b0 = bg * BGRP + bh * MMB
                                nc.tensor.matmul(
                                    po[:, bh * MMB:(bh + 1) * MMB],
                                    w_sb[:, wi, :],          # lhsT [i, o]
                                    xs[ci][:, b0:b0 + MMB],
                                    start=(kc == 0),
                                    stop=(kc == KC - 1),
                                )
                        # Out tiles span XGRP batch columns so out-DMA lines
                        # stay >=4KB even at 2-byte output dtypes.
                        if bg % NXI == 0:
                            ot = opool.tile(
                                [P, XGRP], out_dt, tag=f"ot{d}_{oc}",
                                name=f"ot{d}_{oc}")
                        otv = ot[:, (bg % NXI) * BGRP:(bg % NXI + 1) * BGRP]
                        # Split each evacuation across DVE and ACT so PSUM
                        # banks free sooner and both engines share the load.
                        nc.vector.tensor_copy(otv[:, :MMB], po[:, :MMB])
                        nc.scalar.copy(otv[:, MMB:], po[:, MMB:])
                        if bg % NXI == NXI - 1:
                            row0 = d * BLK + oc * P
                            nc.sync.dma_start(
                                outT[row0:row0 + P,
                                     (bg - NXI + 1) * BGRP:
                                     (bg + 1) * BGRP],
                                ot[:])
    nc.compile()
    return nc


def _np_dt(name):
    if name == "bfloat16":
        import ml_dtypes
        return np.dtype(ml_dtypes.bfloat16)
    return np.dtype(np.float32)


def _shard_inputs(x: np.ndarray, W: np.ndarray):
    """Per-core input maps: transposed x slice + transposed diag W chunks."""
    io_dt = _np_dt(MM_DTYPE)
    in_maps = []
    for c in range(NCORES):
        xs = x[:, c * FPC:(c + 1) * FPC]
        xT = np.ascontiguousarray(xs.T).astype(io_dt)        # [1024, 4096]
        wblk = np.empty((P, DPC * KC * OC, P), np.float32)   # [i, (d,kc,oc), o]
        for d in range(DPC):
            gd = c * DPC + d
            Wb = W[gd * BLK:(gd + 1) * BLK, gd * BLK:(gd + 1) * BLK]  # [o, i]
            for kc in range(KC):
                for oc in range(OC):
                    wblk[:, (d * KC + kc) * OC + oc, :] = \
                        Wb[oc * P:(oc + 1) * P, kc * P:(kc + 1) * P].T
        in_maps.append({"xT": xT, "wT": np.ascontiguousarray(
            wblk.reshape(P, DPC * KC * OC * P)).astype(io_dt)})
    return in_maps


def _unshard_output(results):
    out = np.empty((B, NCORES * FPC), np.float32)
    for c in range(NCORES):
        out[:, c * FPC:(c + 1) * FPC] = results[c]["outT"].T.astype(np.float32)
    return out


_NC_CACHE = None


def kernel(x: np.ndarray, W: np.ndarray) -> np.ndarray:
    import time
    from concourse.bass_utils import run_bass_kernel_spmd

    global _NC_CACHE
    if _NC_CACHE is None:
        _NC_CACHE = _build_bass()
    in_maps = _shard_inputs(np.asarray(x, np.float32), np.asarray(W, np.float32))
    last_exc = None
    for attempt in range(3):
        try:
            res = run_bass_kernel_spmd(
                _NC_CACHE, in_maps, core_ids=list(range(NCORES)))
            return _unshard_output(res.results)
        except Exception as e:  # transient NRT/device errors: retry
            last_exc = e
            time.sleep(2.0)
    raise last_exc


# revision 35
# speedup vs baseline: 1.1585x; 1.1446x over previous
"""Trainium2 Bass kernel for nn_EnsembleLinear (block-diagonal ensemble linear).

Full-input contract: kernel(x, W) -> out with
    x  [4096, 8192] f32
    W  [8192, 8192] f32 (only the 32 diagonal 256x256 blocks matter)
    out[4096, 8192] f32,  out[b, d*256+o] = sum_i W[d*256+o, d*256+i] * x[b, d*256+i]

Sharding: detectors (block axis) split across 8 cores, 4 detectors/core.
Host prep transposes each core's x slice and extracts/transposes the W
diagonal blocks so the device kernel is a pure stationary-weight GEMM
with no on-chip transposes. Output is produced transposed ([o, b]) and
un-transposed on the host.
"""

import numpy as np

B = 4096          # batch
NCORES = 8
DPC = 4           # detectors per core
BLK = 256         # block size (both in/out features per detector)
P = 128           # partitions
KC = BLK // P     # k chunks per block (2)
OC = BLK // P     # o chunks per block (2)
MMB = 512         # batch columns per matmul (max fp32 moving free dim)
BGRP = 1024       # batch columns per compute stage
XGRP = 4096       # batch columns per out tile (8KB bf16 lines)
NBH = BGRP // MMB # matmul slices per stage
NBG = B // BGRP   # compute groups
NXI = XGRP // BGRP  # compute groups per out tile
FPC = DPC * BLK   # features per core (1024)

# Matmul input dtype: "float32" (exact, 4 cyc/row), "float32r" (1 cyc/row),
# or "bfloat16" (1 cyc/row, halves input DMA bytes).
MM_DTYPE = "bfloat16"
# Output DMA dtype: "float32" (exact) or "bfloat16" (halves output DMA bytes).
OUT_DTYPE = "bfloat16"


def _build_bass():
    import concourse.mybir as mybir
    import concourse.tile as tile
    from concourse import bacc

    mm_dt = getattr(mybir.dt, MM_DTYPE)
    out_dt = getattr(mybir.dt, OUT_DTYPE)
    nc = bacc.Bacc("TRN2", target_bir_lowering=False, debug=False,
                   num_devices=NCORES)
    xT = nc.dram_tensor("xT", [FPC, B], mm_dt, kind="ExternalInput")
    wT = nc.dram_tensor("wT", [P, DPC * KC * OC * P], mm_dt, kind="ExternalInput")
    outT = nc.dram_tensor("outT", [FPC, B], out_dt, kind="ExternalOutput")

    with tile.TileContext(nc) as tc:
        with (
            tc.tile_pool(name="wpool", bufs=1) as wpool,
            tc.tile_pool(name="xpool", bufs=1) as xpool,
            tc.tile_pool(name="opool", bufs=1) as opool,
            tc.tile_pool(name="psum", bufs=4, space="PSUM") as psum,
        ):
            # Stationary weights, resident for the whole kernel:
            # w_sb[i, (d,kc,oc), o]
            w_sb = wpool.tile([P, DPC * KC * OC, P], mm_dt)
            nc.sync.dma_start(w_sb[:], wT.rearrange("p (c o) -> p c o", o=P))

            # Load ALL of x up front (8KB/partition lines, one DMA per
            # i-chunk). The whole per-core x fits in SBUF at 2-byte dtypes,
            # so no slab pipelining is needed and the PE never waits on a
            # mid-kernel input stage.
            xT_v = xT.rearrange("(c p) b -> p c b", p=P)  # [128, 8, 4096]
            xs = []
            for ci in range(DPC * KC):
                xc = xpool.tile([P, B], mm_dt, tag=f"xc{ci}")
                nc.sync.dma_start(xc[:], xT_v[:, ci, :])
                xs.append(xc)
            # d-major compute order: detector d only needs x chunks 2d and
            # 2d+1, which are exactly the chunks that have already arrived
            # when its matmuls are reached — the PE never waits mid-kernel.
            for d in range(DPC):
                for oc in range(OC):
                    ot = None
                    for bg in range(NBG):
                        po = psum.tile([P, BGRP], mybir.dt.float32)
                        for kc in range(KC):
                            ci = d * KC + kc
                            wi = (d * KC + kc) * OC + oc
                            for bh in range(NBH):
                                b0 = bg * BGRP + bh * MMB
                                nc.tensor.matmul(
                                    po[:, bh * MMB:(bh + 1) * MMB],
                                    w_sb[:, wi, :],          # lhsT [i, o]
                                    xs[ci][:, b0:b0 + MMB],
                                    start=(kc == 0),
                                    stop=(kc == KC - 1),
                                )
                        # Out tiles span XGRP batch columns so out-DMA lines
                        # stay >=4KB even at 2-byte output dtypes.
                        if bg % NXI == 0:
                            ot = opool.tile(
                                [P, XGRP], out_dt, tag=f"ot{d}_{oc}",
                                name=f"ot{d}_{oc}")
                        otv = ot[:, (bg % NXI) * BGRP:(bg % NXI + 1) * BGRP]
                        # Alternate whole-tile evacuations between DVE and ACT
                        # (fewer instructions/semaphores than split halves).
                        if (d * OC * NBG + oc * NBG + bg) % 2 == 0:
                            nc.vector.tensor_copy(otv[:], po[:])
                        else:
                            nc.scalar.copy(otv[:], po[:])
                        if bg % NXI == NXI - 1:
                            row0 = d * BLK + oc * P
                            nc.sync.dma_start(
                                outT[row0:row0 + P,
                                     (bg - NXI + 1) * BGRP:
                                     (bg + 1) * BGRP],
                                ot[:])
    nc.compile()
    return nc


def _np_dt(name):
    if name == "bfloat16":
        import ml_dtypes
        return np.dtype(ml_dtypes.bfloat16)
    return np.dtype(np.float32)


def _shard_inputs(x: np.ndarray, W: np.ndarray):
    """Per-core input maps: transposed x slice + transposed diag W chunks."""
    io_dt = _np_dt(MM_DTYPE)
    in_maps = []
    for c in range(NCORES):
        xs = x[:, c * FPC:(c + 1) * FPC]
        xT = np.ascontiguousarray(xs.T).astype(io_dt)        # [1024, 4096]
        wblk = np.empty((P, DPC * KC * OC, P), np.float32)   # [i, (d,kc,oc), o]
        for d in range(DPC):
            gd = c * DPC + d
            Wb = W[gd * BLK:(gd + 1) * BLK, gd * BLK:(gd + 1) * BLK]  # [o, i]
            for kc in range(KC):
                for oc in range(OC):
                    wblk[:, (d * KC + kc) * OC + oc, :] = \
                        Wb[oc * P:(oc + 1) * P, kc * P:(kc + 1) * P].T
        in_maps.append({"xT": xT, "wT": np.ascontiguousarray(
            wblk.reshape(P, DPC * KC * OC * P)).astype(io_dt)})
    return in_maps


def _unshard_output(results):
    out = np.empty((B, NCORES * FPC), np.float32)
    for c in range(NCORES):
        out[:, c * FPC:(c + 1) * FPC] = results[c]["outT"].T.astype(np.float32)
    return out


_NC_CACHE = None


def kernel(x: np.ndarray, W: np.ndarray) -> np.ndarray:
    import time
    from concourse.bass_utils import run_bass_kernel_spmd

    global _NC_CACHE
    if _NC_CACHE is None:
        _NC_CACHE = _build_bass()
    in_maps = _shard_inputs(np.asarray(x, np.float32), np.asarray(W, np.float32))
    last_exc = None
    for attempt in range(3):
        try:
            res = run_bass_kernel_spmd(
                _NC_CACHE, in_maps, core_ids=list(range(NCORES)))
            return _unshard_output(res.results)
        except Exception as e:  # transient NRT/device errors: retry
            last_exc = e
            time.sleep(2.0)
    raise last_exc
